# revision 27
# baseline (speedup 1.0000x reference)
"""BlendShapes model kernel for 8 Trainium2 NeuronCores.

Computation (reference):
    pose_repr = pose[:, 1:].reshape(B, 23, 9) - eye      # (B, J, 9)
    per-joint MLP 9 -> 18 -> 32 -> 8 (ReLU between)      # coff (B, J, 8)
    basis_full = basis[:, None] * mask[:, :, None, None]  # (V, J, 8, 3)
    res = einsum('bjk,vjkc->bvc', coff, basis_full)       # (B, V, 3)

Mapping (2-way batch x 4-way vertex shard):
  - Core i handles batch half i//4 (512 rows) and vertex quarter i%4
    (1728 of 6912 padded vertices). Halving batch per core halves the
    (otherwise replicated) MLP work; the main-matmul and store volumes
    per core are unchanged.
  - basis*mask is folded on the host into one bf16 tensor bfm with an
    exact 2^13 scale; PSUM evacuation applies 2^-13 while widening to f32.
  - MLP activations are transposed ([feat, batch]) so coff^T feeds the
    main matmul's stationary operand directly. Joints are packed
    block-diagonally in chunks of 4 (tail 3); L3 chunk outputs stack at
    32-aligned partition offsets of two PSUM tiles (bases 0/32/64), so
    coffT_a (joints 0-11, K=96) / coffT_b (joints 12-22, K=88) come from
    single whole-tile epilogues. Main K split is 96 + 88.
  - Main matmul: 8 virtual tiles (4 batch-tiles x 2 column halves of
    5184); per tile, N pairs share a 2-bank PSUM tile, K-pass-outer so
    the stationary operand is reused across the 3 pairs; evacuations
    alternate ACT/DVE; stores all go on the sync HWDGE ring (the scalar
    ring only carries input loads, keeping ACT free for epilogues).
"""

import numpy as np
import ml_dtypes

BF16 = ml_dtypes.bfloat16

N_VERT, N_JOINT, BPJ, BATCH = 6890, 23, 8, 1024
VPAD = 6912
BC = BATCH // 2  # 512 batch rows per core
VC = VPAD // 4  # 1728 vertices per core
VC3 = VC * 3  # 5184 columns per core
VH = VC3 // 2  # 2592, virtual-tile column extent
KTOT = N_JOINT * BPJ  # 184
NT_BOUNDS = [0, 512, 1024, 1536, 2048, 2560, 2592]
NT_PAIRS = [(0, 1), (2, 3), (4, 5)]
NBT = BC // 128  # 4 batch tiles per core

CHUNKS = [(0, 4), (4, 8), (8, 12), (12, 16), (16, 20), (20, 23)]


def _offsets(mpj):
    offs, col = [], 0
    for js, je in CHUNKS:
        offs.append(col)
        col += (je - js) * mpj
    return offs, col


W1_OFF, W1_TOT = _offsets(18)  # 414
W2_OFF, W2_TOT = _offsets(32)  # 736
W3_OFF, W3_TOT = _offsets(8)   # 184
W2_OFF = [W1_TOT + o for o in W2_OFF]
W3_OFF = [W1_TOT + W2_TOT + o for o in W3_OFF]
W_COLS = W1_TOT + W2_TOT + W3_TOT  # 1334

# bias_all columns: [0:6] L1 bias, [6:12] L2 bias, [12] L3 bias rows for
# coffT_a (joints 0-11, j-major), [13] same for coffT_b (joints 12-22),
# [14:20] eye vectors per chunk (for pose_repr = pose - eye).
BIAS_COLS = 20
BSCALE = 8192.0  # 2**13
DESCALE = 1.0 / 8192.0

_CACHED = {}


def _build_nc():
    import concourse.tile as tile
    from concourse import bacc, mybir
    from contextlib import ExitStack

    dt = mybir.dt
    f32, f16 = dt.float32, dt.bfloat16
    AF = mybir.ActivationFunctionType
    ALU = mybir.AluOpType

    nc = bacc.Bacc(None, target_bir_lowering=False)

    pose_t = nc.dram_tensor("pose_t", [207, BC], f16, kind="ExternalInput")
    bfm_t = nc.dram_tensor("bfm_t", [KTOT, VC3], f16, kind="ExternalInput")
    w_all = nc.dram_tensor("w_all", [128, W_COLS], f16, kind="ExternalInput")
    bias_all = nc.dram_tensor("bias_all", [128, BIAS_COLS], f32, kind="ExternalInput")
    res = nc.dram_tensor("res", [BC, VC3], f32, kind="ExternalOutput")

    with ExitStack() as ctx:
        tc = ctx.enter_context(tile.TileContext(nc))
        const = ctx.enter_context(tc.tile_pool(name="const", bufs=1))
        work = ctx.enter_context(tc.tile_pool(name="work", bufs=1))
        outp = ctx.enter_context(tc.tile_pool(name="outp", bufs=4))

        # ---- input DMAs, split across both HWDGE rings so the MLP-critical
        # pieces (bias, weights, early pose chunks) land fast and in parallel.
        # All HWDGE loads go on the sync ring (SP has nothing else to do
        # until the stores); the scalar (ACT) queue must stay free for MLP
        # epilogues -- a DMA-issue instruction there blocks them. The tiny
        # bias load (128 x 80 B descriptors, slow to drain) rides the
        # otherwise-idle GPSIMD (SWDGE) ring.
        bfm_a = work.tile([96, VC3], f16, tag="bfm_a")
        bfm_b = work.tile([88, VC3], f16, tag="bfm_b")
        pose_c = [None] * 6

        def pose_load(eng, c):
            K = 9 * (CHUNKS[c][1] - CHUNKS[c][0])
            r0 = 9 * CHUNKS[c][0]
            t = work.tile([K, BC], f16, tag=f"pose_{c}", name=f"pose_{c}")
            eng.dma_start(out=t[:], in_=pose_t[r0 : r0 + K, :])
            pose_c[c] = t

        # sync ring: w + odd pose chunks. scalar ring: even pose chunks,
        # then bfm -- gated by an artificial dependency on the last pose
        # chunk so its fat packets cannot starve the small loads (the
        # scheduler hoists ready DMA issues; only data deps sequence them).
        # w in two same-ring DMAs: L1 only waits the first 414 columns.
        w_sb = const.tile([128, W_COLS], f16, tag="w")
        nc.sync.dma_start(out=w_sb[:, 0:W1_TOT], in_=w_all[:, 0:W1_TOT])
        nc.sync.dma_start(out=w_sb[:, W1_TOT:W_COLS], in_=w_all[:, W1_TOT:W_COLS])
        bias_sb = const.tile([128, BIAS_COLS], f32, tag="bias")
        nc.gpsimd.dma_start(out=bias_sb[:], in_=bias_all[:, :])
        for c in (0, 2, 4):
            pose_load(nc.scalar, c)
        for c in (1, 3, 5):
            pose_load(nc.sync, c)
        nc.vector.tensor_scalar(
            out=bfm_b[0:1, 0:1], in0=pose_c[2][0:1, 0:1], scalar1=0.0,
            scalar2=None, op0=ALU.mult,
        )
        nc.sync.dma_start(out=bfm_a[:], in_=bfm_t[0:96, :])
        nc.scalar.dma_start(out=bfm_b[:], in_=bfm_t[96:KTOT, :])

        coffT_a = work.tile([96, BC], f16, tag="coffT_a")
        coffT_b = work.tile([88, BC], f16, tag="coffT_b")
        h1 = {}
        h2 = {}
        ep_ctr = [0]

        def epilogue(dst, ps, bias_ap, relu):
            use_act = ep_ctr[0] % 2 == 0
            ep_ctr[0] += 1
            if relu and use_act:
                nc.scalar.activation(dst, ps, AF.Relu, bias=bias_ap)
            elif relu:
                nc.vector.tensor_scalar(
                    out=dst, in0=ps, scalar1=bias_ap, scalar2=0.0,
                    op0=ALU.add, op1=ALU.max,
                )
            else:
                nc.vector.tensor_scalar(
                    out=dst, in0=ps, scalar1=bias_ap, scalar2=None, op0=ALU.add
                )

        # ---- MLP over this core's 512 batch columns, software-pipelined:
        # interleave L1/L2/L3 chunk matmuls so each epilogue's latency hides
        # under other chunks' PE streams.
        def l1(pmlp, c):
            js, je = CHUNKS[c]
            nj = je - js
            K, M = 9 * nj, 18 * nj
            off = W1_OFF[c]
            ps = pmlp.tile([M, BC], f32, tag="psmlp", name=f"ps1_{c}")
            nc.tensor.matmul(
                ps[:], lhsT=w_sb[0:K, off : off + M], rhs=pose_c[c][:],
                start=True, stop=True,
            )
            h1[c] = work.tile([M, BC], f16, tag=f"h1_{c}", name=f"h1_{c}")
            epilogue(h1[c][:], ps[:], bias_sb[0:M, c : c + 1], True)

        def l2(pmlp, c):
            js, je = CHUNKS[c]
            nj = je - js
            K, M = 18 * nj, 32 * nj
            off = W2_OFF[c]
            ps = pmlp.tile([M, BC], f32, tag="psmlp", name=f"ps2_{c}")
            nc.tensor.matmul(
                ps[:], lhsT=w_sb[0:K, off : off + M], rhs=h1[c][:],
                start=True, stop=True,
            )
            h2[c] = work.tile([M, BC], f16, tag=f"h2_{c}", name=f"h2_{c}")
            epilogue(h2[c][:], ps[:], bias_sb[0:M, 6 + c : 7 + c], True)

        def l3(pmlp, c, ps3a, ps3b):
            js, je = CHUNKS[c]
            nj = je - js
            K, M = 32 * nj, 8 * nj
            off = W3_OFF[c]
            if c < 3:
                dst = ps3a[32 * c : 32 * c + M, :]
            else:
                dst = ps3b[32 * (c - 3) : 32 * (c - 3) + M, :]
            nc.tensor.matmul(
                dst[:], lhsT=w_sb[0:K, off : off + M], rhs=h2[c][:],
                start=True, stop=True,
            )

        with tc.tile_pool(name="pmlp", bufs=3, space="PSUM") as pmlp, \
                tc.tile_pool(name="pl3", bufs=1, space="PSUM") as pl3:
            l1(pmlp, 0)
            l1(pmlp, 2)
            l1(pmlp, 4)
            l2(pmlp, 0)
            l1(pmlp, 1)
            l2(pmlp, 2)
            l1(pmlp, 3)
            l2(pmlp, 4)
            l1(pmlp, 5)
            l2(pmlp, 1)
            ps3a = pl3.tile([96, BC], f32, tag="ps3a", name="ps3a")
            ps3b = pl3.tile([88, BC], f32, tag="ps3b", name="ps3b")
            l3(pmlp, 0, ps3a, ps3b)
            l3(pmlp, 1, ps3a, ps3b)
            l2(pmlp, 3)
            l3(pmlp, 2, ps3a, ps3b)
            epilogue(coffT_a[:], ps3a[:], bias_sb[0:96, 12:13], False)
            l2(pmlp, 5)
            l3(pmlp, 3, ps3a, ps3b)
            l3(pmlp, 4, ps3a, ps3b)
            l3(pmlp, 5, ps3a, ps3b)
            epilogue(coffT_b[:], ps3b[:], bias_sb[0:88, 13:14], False)

        # ---- main matmul over 8 virtual tiles (bt x column half).
        with tc.tile_pool(name="pmain", bufs=4, space="PSUM") as pmain:
            vtiles = [(bt, vh) for bt in range(NBT) for vh in (0, 1)]
            for vi, (bt, vh) in enumerate(vtiles):
                bsl = slice(bt * 128, (bt + 1) * 128)
                v0 = vh * VH
                ostrip = outp.tile(
                    [128, VH], f32, tag="ostrip", name=f"o_{bt}_{vh}"
                )
                pstiles = [
                    pmain.tile([128, 1024], f32, tag="ps", name=f"ps_{vi}_{p}")
                    for p in range(len(NT_PAIRS))
                ]
                for ki, (cof, bfm, kk) in enumerate(
                    ((coffT_a, bfm_a, 96), (coffT_b, bfm_b, 88))
                ):
                    for p, (t0, t1) in enumerate(NT_PAIRS):
                        s0 = slice(v0 + NT_BOUNDS[t0], v0 + NT_BOUNDS[t0 + 1])
                        s1 = slice(v0 + NT_BOUNDS[t1], v0 + NT_BOUNDS[t1 + 1])
                        n1 = NT_BOUNDS[t1 + 1] - NT_BOUNDS[t1]
                        ps = pstiles[p]
                        nc.tensor.matmul(
                            ps[:, 0:512], lhsT=cof[:, bsl], rhs=bfm[0:kk, s0],
                            start=ki == 0, stop=ki == 1,
                        )
                        nc.tensor.matmul(
                            ps[:, 512 : 512 + n1], lhsT=cof[:, bsl],
                            rhs=bfm[0:kk, s1], start=ki == 0, stop=ki == 1,
                        )
                for p, (t0, t1) in enumerate(NT_PAIRS):
                    n1 = NT_BOUNDS[t1 + 1] - NT_BOUNDS[t1]
                    osl = slice(NT_BOUNDS[t0], NT_BOUNDS[t0] + 512 + n1)
                    ps = pstiles[p]
                    if ep_ctr[0] % 2 == 0:
                        nc.scalar.activation(
                            ostrip[:, osl], ps[:, 0 : 512 + n1], AF.Copy,
                            scale=DESCALE,
                        )
                    else:
                        nc.vector.tensor_scalar(
                            out=ostrip[:, osl], in0=ps[:, 0 : 512 + n1],
                            scalar1=DESCALE, scalar2=None, op0=ALU.mult,
                        )
                    ep_ctr[0] += 1
                # full-tile stores alternate rings; the final tile is
                # half-split across both so the trailing drain is short.
                if vi == len(vtiles) - 1:
                    nc.sync.dma_start(
                        out=res[bsl, v0 : v0 + VH // 2],
                        in_=ostrip[:, 0 : VH // 2],
                    )
                    nc.scalar.dma_start(
                        out=res[bsl, v0 + VH // 2 : v0 + VH],
                        in_=ostrip[:, VH // 2 : VH],
                    )
                elif vi % 2 == 0:
                    nc.sync.dma_start(out=res[bsl, v0 : v0 + VH], in_=ostrip[:])
                else:
                    nc.scalar.dma_start(
                        out=res[bsl, v0 : v0 + VH], in_=ostrip[:]
                    )

    nc.finalize()
    return nc


def _pack_host(pose, basis, mask, w1, b1, w2, b2, w3, b3):
    pose_tt = np.ascontiguousarray(
        pose[:, 1:].reshape(BATCH, 207).T.astype(BF16)
    )  # [207, B] rows are (j, i)

    # bfm[j*8+k, v*3+c] = basis[v, k, c] * mask[v, j] * 2^13
    prod = (
        basis[None, :, :, :] * mask.T[:, :, None, None] * BSCALE
    )  # (J, V, 8, 3) f32
    bfm = np.zeros((KTOT, VPAD * 3), BF16)
    bfm[:, : N_VERT * 3] = prod.transpose(0, 2, 1, 3).reshape(KTOT, N_VERT * 3)

    w_pack = np.zeros((128, W_COLS), BF16)
    bias_all = np.zeros((128, BIAS_COLS), np.float32)
    for (js, je), o1, o2, o3 in zip(CHUNKS, W1_OFF, W2_OFF, W3_OFF):
        for t, j in enumerate(range(js, je)):
            w_pack[t * 9 : (t + 1) * 9, o1 + t * 18 : o1 + (t + 1) * 18] = w1[j]
            w_pack[t * 18 : (t + 1) * 18, o2 + t * 32 : o2 + (t + 1) * 32] = w2[j]
            w_pack[t * 32 : (t + 1) * 32, o3 + t * 8 : o3 + (t + 1) * 8] = w3[j]
    b1f = b1 - w1[:, 0] - w1[:, 4] - w1[:, 8]  # fold pose-eye into L1 bias
    for c, (js, je) in enumerate(CHUNKS):
        nj = je - js
        bias_all[0 : 18 * nj, c] = b1f[js:je].reshape(-1)
        bias_all[0 : 32 * nj, 6 + c] = b2[js:je].reshape(-1)
    bias_all[0:96, 12] = b3[0:12].reshape(-1)
    bias_all[0:88, 13] = b3[12:23].reshape(-1)

    return pose_tt, bfm, w_pack, bias_all


def _in_maps(pose, basis, mask, w1, b1, w2, b2, w3, b3):
    pose_tt, bfm, w_pack, bias_all = _pack_host(
        np.asarray(pose, np.float32),
        np.asarray(basis, np.float32),
        np.asarray(mask, np.float32),
        np.asarray(w1, np.float32),
        np.asarray(b1, np.float32),
        np.asarray(w2, np.float32),
        np.asarray(b2, np.float32),
        np.asarray(w3, np.float32),
        np.asarray(b3, np.float32),
    )
    maps = []
    for i in range(8):
        bh, vq = i // 4, i % 4
        maps.append(
            {
                "pose_t": np.ascontiguousarray(
                    pose_tt[:, bh * BC : (bh + 1) * BC]
                ),
                "bfm_t": np.ascontiguousarray(
                    bfm[:, vq * VC3 : (vq + 1) * VC3]
                ),
                "w_all": w_pack,
                "bias_all": bias_all,
            }
        )
    return maps


def kernel(pose, basis, mask, w1, b1, w2, b2, w3, b3):
    from concourse.bass_utils import run_bass_kernel_spmd

    if "nc" not in _CACHED:
        _CACHED["nc"] = _build_nc()
    nc = _CACHED["nc"]

    maps = _in_maps(pose, basis, mask, w1, b1, w2, b2, w3, b3)
    r = run_bass_kernel_spmd(nc, maps, core_ids=list(range(8)))
    full = np.empty((BATCH, VPAD * 3), np.float32)
    for i in range(8):
        bh, vq = i // 4, i % 4
        full[bh * BC : (bh + 1) * BC, vq * VC3 : (vq + 1) * VC3] = r.results[i][
            "res"
        ]
    out = full.reshape(BATCH, VPAD, 3)
    return np.ascontiguousarray(out[:, :N_VERT, :])
